# revision 75
# baseline (speedup 1.0000x reference)
"""Multi-head attention (nn_GroupQueryAttention_163208757512) on 8 TRN2 cores.

Problem: B=2, S=2048, D=1024, H=16 heads, DK=64. f32 in/out.
    q = Q @ Wq.T + bq  (per head)   k, v likewise
    out = softmax(q k^T / 8) v -> concat heads -> @ Wo.T + bo

Sharding: core c handles batch b=c//4 and head group g=c%4 (4 heads,
feature slice hs = 256*g : 256*g+256). Data parallel on B, tensor
parallel on heads; the output projection yields per-core partials that
the host sums (replaces the all-reduce).

Device-side design (v4 — one ScalarE-exp-bound software pipeline):
  - the kernel is one long stream of (score -> exp -> PV) iterations,
    ScalarE-bound (~1.1us/exp tile); ALL other PE work (v/q projections
    for later chunks, x_norm transposes, output-projection tiles) is
    chopped into ~1us jobs and popped one-per-iteration from a FIFO so
    the in-order PE queue never starves ScalarE.
  - prologue projects only what the first pass needs: k (all), v+q of
    chunk 0. DMA stream order matches consumption (wk, kx+vx half 0,
    kx half 1, qx half 0, vx half 1, qx half 1, then wo/ident/bo).
  - all matmuls bf16 (1 cycle/row @ 2.4GHz, FWL); f32 PSUM.
  - scores pre-transposed: S^T[sk, sq] = K_h Q_h^T, two heads
    row-packed via tile_position; exp on ScalarE from 2-bank PSUM,
    scale=1/8, bf16 out, issued one sk-tile ahead of PV (skew).
  - PV is p-stationary: lhsT = pt[sk, sq-tile], rhs = [v_h | 1] (ones
    column accumulates softmax denominators) -> xaug[sq, 4, 65]; the
    denominator is per-partition so normalize is a [P,4] reciprocal +
    tensor_scalar_mul (no cross-partition broadcast).
  - x_norm [sq, dh] transposes back to [dh, s] via PE transpose-mode.
  - biases: bq/bk via per-partition tensor_scalar on evacuation; bv and
    bo as rank-1 (ones x bias) f32r matmuls appended to the v / y
    accumulation groups.
  - PSUM: one 3-pool layout for the whole program — sp 2x2 banks
    (scores, and borrowed by projection jobs), xaug 2x1, misc 2x1
    (transpose-out + y tiles). start=True clears has_written for the
    WHOLE bank, so bank-sharing accumulation groups only set start on
    the first group per bank.

Constraint discovered on this toolchain: walrus allows ONE sync-wait per
instruction, so a post-pass (split_waits) chains excess waits onto NoOps.
Custom DVE ops and ALU-divide are rejected by this walrus build.
"""

import numpy as np
from collections import deque
from contextlib import ExitStack

import ml_dtypes

import concourse.bass as bass
import concourse.mybir as mybir
import concourse.tile as tile
from concourse.bass import ds, ts
from concourse.bass_utils import run_bass_kernel_spmd

F32 = mybir.dt.float32
F32R = mybir.dt.float32r
BF16 = mybir.dt.bfloat16
AF = mybir.ActivationFunctionType
ALU = mybir.AluOpType
BF_NP = ml_dtypes.bfloat16

B, S, D, H = 2, 2048, 1024, 16
DK = D // H            # 64
NCORES = 8
GROUPS = 4             # head groups per batch
DH = D // GROUPS       # 256 feature cols per core
P = 128
KT = D // P            # 8 contraction tiles for projections
ST = S // P            # 16 s-tiles
CH = 4                 # s-chunks
CW = S // CH           # 512
SW = 2 * CW            # 1024-wide DMA staging (2KB bf16 lines)
HALVES = S // SW       # 2


# ---------------------------------------------------------------- wait fix
_wf_counter = [0]


def _split_waits(nc, cap=1):
    """walrus in this container accepts at most one sync-wait command per
    instruction; chain the rest onto same-engine NoOps placed just before."""
    for fn in nc.m.functions:
        for bb in fn.blocks:
            out, changed = [], False
            for inst in bb.instructions:
                si = inst.sync_info
                waits = list(si.on_wait) if (si is not None and si.on_wait) else []
                if len(waits) > cap:
                    changed = True
                    keep = waits[-cap:]
                    for i in range(0, len(waits) - cap, cap):
                        _wf_counter[0] += 1
                        out.append(mybir.InstNoOp(
                            name=f"waitfix_{_wf_counter[0]}",
                            sync_info=mybir.SyncInfo(
                                on_wait=waits[i:i + cap], on_update=[]),
                            engine=inst.engine,
                            bass_nofuse=True,
                        ))
                    inst.sync_info = mybir.SyncInfo(
                        on_wait=keep,
                        on_update=list(si.on_update) if si else [])
                out.append(inst)
            if changed:
                bb.instructions = out
    return nc


# ---------------------------------------------------------------- program
def build_program(apply_waitfix=True):
    nc = bass.Bass()

    xqt = nc.dram_tensor("xqt", [D, S], BF16, kind="ExternalInput")
    xkt = nc.dram_tensor("xkt", [D, S], BF16, kind="ExternalInput")
    xvt = nc.dram_tensor("xvt", [D, S], BF16, kind="ExternalInput")
    wq_h = nc.dram_tensor("wq_h", [P, KT * DH], BF16, kind="ExternalInput")
    wk_h = nc.dram_tensor("wk_h", [P, KT * DH], BF16, kind="ExternalInput")
    wv_h = nc.dram_tensor("wv_h", [P, KT * DH], BF16, kind="ExternalInput")
    wo_h = nc.dram_tensor("wo_h", [P, 2 * D], BF16, kind="ExternalInput")
    bq2 = nc.dram_tensor("bq2", [P, 2], F32, kind="ExternalInput")
    bk2 = nc.dram_tensor("bk2", [P, 2], F32, kind="ExternalInput")
    bvr = nc.dram_tensor("bvr", [1, DH], F32R, kind="ExternalInput")
    bo_eff = nc.dram_tensor("bo_eff", [1, D], F32R, kind="ExternalInput")
    onesd = nc.dram_tensor("onesd", [1, P], F32R, kind="ExternalInput")
    identd = nc.dram_tensor("identd", [P, P], BF16, kind="ExternalInput")
    y = nc.dram_tensor("y", [S, D], F32, kind="ExternalOutput")

    xqt_r = xqt.rearrange("(kt p) s -> kt p s", p=P)
    xkt_r = xkt.rearrange("(kt p) s -> kt p s", p=P)
    xvt_r = xvt.rearrange("(kt p) s -> kt p s", p=P)
    y_r = y.rearrange("(st p) d -> st p d", p=P)

    with tile.TileContext(nc) as tc:
      with ExitStack() as ctx:
        # ---- persistent SBUF ----
        wp = ctx.enter_context(tc.tile_pool(name="wp", bufs=1))
        wq_sb = wp.tile([P, KT, DH], BF16, tag="wq")
        wk_sb = wp.tile([P, KT, DH], BF16, tag="wk")
        wv_sb = wp.tile([P, KT, DH], BF16, tag="wv")
        wo_sb = wp.tile([P, 2, D], BF16, tag="wo")
        bq_sb = wp.tile([P, 2], F32, tag="bq")
        bk_sb = wp.tile([P, 2], F32, tag="bk")
        bvr_sb = wp.tile([1, DH], F32R, tag="bvr")
        ones1 = wp.tile([1, P], F32R, tag="ones1")
        bo_sb = wp.tile([1, D], F32R, tag="bo")
        ident = wp.tile([P, P], BF16, tag="ident")

        qt_sb = wp.tile([P, 2, S], BF16, tag="qt")
        kt_sb = wp.tile([P, 2, S], BF16, tag="kt")
        pvw_sb = wp.tile([P, ST, GROUPS, DK + 1], BF16, tag="pvw")
        xn_sb = wp.tile([P, 2, S], BF16, tag="xn")

        # critical-path weight/bias DMAs; everything else is queued after
        # the x streams below, ordered by first use
        # big weights on the sync queue; everything small (and the
        # late-deadline weights) on the idle GpSimd queue so the kx
        # stream starts as early as possible
        nc.sync.dma_start(wk_sb[:], wk_h.rearrange("p (kt m) -> p kt m", kt=KT))
        nc.sync.dma_start(wq_sb[:], wq_h.rearrange("p (kt m) -> p kt m", kt=KT))
        nc.gpsimd.dma_start(bq_sb[:], bq2[:])
        nc.gpsimd.dma_start(bk_sb[:], bk2[:])
        nc.gpsimd.dma_start(bvr_sb[:], bvr[:])
        nc.gpsimd.dma_start(ones1[:], onesd[:])
        nc.gpsimd.dma_start(wv_sb[:],
                            wv_h.rearrange("p (kt m) -> p kt m", kt=KT))
        nc.gpsimd.dma_start(wo_sb[:],
                            wo_h.rearrange("p (p2 d) -> p p2 d", p2=2))
        nc.gpsimd.dma_start(bo_sb[:], bo_eff[:])
        nc.gpsimd.dma_start(ident[:], identd[:])
        nc.vector.memset(pvw_sb[:, :, :, DK:DK + 1], 1.0)
        # dummy exp: pull the ~1.3us ACT_TABLE_LOAD off the critical path
        # (it otherwise fires at the first real exp, ~36us in)
        dum_i = wp.tile([1, 8], F32, tag="dumi")
        dum_o = wp.tile([1, 8], F32, tag="dumo")
        nc.vector.memset(dum_i[:], 0.0)
        nc.scalar.activation(dum_o[:], dum_i[:], AF.Exp, scale=0.125)

        with nc.allow_low_precision(reason="bf16 matmuls, tol is 2e-2"):
          with (
              tc.tile_pool(name="xs", bufs=16) as xs,
              tc.tile_pool(name="ptp", bufs=10) as ptp,
              tc.tile_pool(name="stgp", bufs=4) as stgp,
              tc.tile_pool(name="rcp", bufs=4) as rcpp,
              tc.tile_pool(name="ev", bufs=6) as ev,
              tc.tile_pool(name="spp", bufs=2, space="PSUM") as sp_ps,
              tc.tile_pool(name="xap", bufs=2, space="PSUM") as xa_ps,
              tc.tile_pool(name="mpp", bufs=2, space="PSUM") as misc_ps,
          ):
            # ---- x staging: DMA stream in consumption order ----
            kx = [None] * KT * HALVES
            vx = [None] * KT * HALVES
            qx = [None] * KT * HALVES

            def xdma(dst, src_r, idx):
                half, kt = divmod(idx, KT)
                t = xs.tile([P, SW], BF16, tag=dst, name=f"{dst}_{idx}")
                nc.sync.dma_start(t[:], src_r[kt, :, ds(SW * half, SW)])
                return t

            for i in range(2 * KT):                   # k both halves
                kx[i] = xdma("xk", xkt_r, i)
            for kt in range(KT):                      # q half 0
                qx[kt] = xdma("xq", xqt_r, kt)
            for kt in range(KT):                      # v half 0
                vx[kt] = xdma("xv", xvt_r, kt)
            for kt in range(KT):                      # v half 1
                vx[KT + kt] = xdma("xv", xvt_r, KT + kt)
            for kt in range(KT):                      # q half 1
                qx[KT + kt] = xdma("xq", xqt_r, KT + kt)

            # ---------------- projection emitters ----------------
            # each returns a list of (cost_us, closure) micro-jobs so the
            # pipeline pump can interleave them without starving ScalarE
            def kq_proj_jobs(c, which):
                """project q or k for chunk c into {q,k}t_sb[:, :, csl]."""
                half, sub = divmod(c, 2)
                ssl = ds(CW * sub, CW)
                w_sb, x_t, out_sb, b_sb = (
                    (wq_sb, qx, qt_sb, bq_sb) if which == "q"
                    else (wk_sb, kx, kt_sb, bk_sb))
                st = {}

                def mk(kts):
                    def run():
                        if "pp" not in st:
                            st["pp"] = sp_ps.tile([P, 2, CW], F32, tag="sp",
                                                  name=f"pp{which}_{c}")
                        pp = st["pp"]
                        for kt in kts:
                            for p in range(2):
                                nc.tensor.matmul(pp[:, p, :],
                                                 w_sb[:, kt, ds(P * p, P)],
                                                 x_t[KT * half + kt][:, ssl],
                                                 start=(kt == 0),
                                                 stop=(kt == KT - 1))
                    return run

                def fin():
                    pp = st["pp"]
                    for p in range(2):
                        nc.vector.tensor_scalar_add(
                            out_sb[:, p, ds(CW * c, CW)],
                            pp[:, p, :], b_sb[:, p:p + 1])

                return [(0.85, mk((0, 1))), (0.85, mk((2, 3))),
                        (0.85, mk((4, 5))), (0.85, mk((6, 7))), (0.15, fin)]

            def v_proj_jobs(c):
                """project v for chunk c into pvw_sb[:, 4c:4c+4, :, 0:DK]."""
                half, sub = divmod(c, 2)
                st = {}

                def mk(kts):
                    def run():
                        if "vp" not in st:
                            st["vp"] = sp_ps.tile([P, 2, CW], F32, tag="sp",
                                                  name=f"ppv_{c}")
                        vp = st["vp"]
                        for kt in kts:
                            for st4 in range(4):
                                j, u = divmod(st4, 2)
                                nc.tensor.matmul(
                                    vp[:, j, ds(DH * u, DH)],
                                    vx[KT * half + kt][
                                        :, ds(CW * sub + P * st4, P)],
                                    wv_sb[:, kt, :],
                                    start=(kt == 0 and u == 0), stop=False)
                    return run

                def fin():
                    vp = st["vp"]
                    for st4 in range(4):
                        j, u = divmod(st4, 2)
                        nc.tensor.matmul(vp[:, j, ds(DH * u, DH)],
                                         ones1[:1, 0:P], bvr_sb[:],
                                         start=False, stop=True)
                    for st4 in range(4):
                        j, u = divmod(st4, 2)
                        nc.vector.tensor_copy(
                            pvw_sb[:, 4 * c + st4, :, 0:DK],
                            vp[:, j, ds(DH * u, DH)].rearrange(
                                "p (h d) -> p h d", h=GROUPS))

                return [(0.85, mk((0, 1))), (0.85, mk((2, 3))),
                        (0.85, mk((4, 5))), (0.85, mk((6, 7))), (0.3, fin)]

            def run_all(jobs):
                for _, fn in jobs:
                    fn()

            # ---------------- deferred-job emitters ----------------
            def transpose_job(c, p, hh, i, stg):
                def run():
                    tp = misc_ps.tile([P, 2 * CW], BF16, tag="m",
                                      name=f"tp_{c}_{p}_{hh}_{i}")
                    blk = stg[:, 2 * i:2 * i + 2, :].rearrange(
                        "p a b -> p (a b)")
                    nc.tensor.transpose(tp[:, 0:P], blk, ident[:])
                    for jj in range(2):
                        m = 2 * i + jj
                        nc.vector.tensor_copy(
                            xn_sb[64 * hh:64 * hh + 64, p,
                                  ds(CW * c + P * m, P)],
                            tp[ds(64 * jj, 64), 0:P])
                return run

            def emit_ymms(yp, st, oc):
                for p2 in range(2):
                    nc.tensor.matmul(yp[:], xn_sb[:, p2, ts(st, P)],
                                     wo_sb[:, p2, ds(CW * oc, CW)],
                                     start=(p2 == 0), stop=False)
                nc.tensor.matmul(yp[:], ones1[:1, 0:P],
                                 bo_sb[:, ds(CW * oc, CW)],
                                 start=False, stop=True)

            def emit_yout(yp, st, oc, direct=False):
                if direct:
                    # tail only: DMA straight from PSUM, skipping the
                    # VectorE evacuation on the serial drain chain
                    nc.sync.dma_start(y_r[st, :, ds(CW * oc, CW)], yp[:])
                    return
                ysb = ev.tile([P, CW], F32, tag="ysb",
                              name=f"ysb_{st}_{oc}")
                nc.vector.tensor_copy(ysb[:], yp[:])
                nc.sync.dma_start(y_r[st, :, ds(CW * oc, CW)], ysb[:])

            def ytile_job(st, oc):
                def run():
                    yp = misc_ps.tile([P, CW], F32, tag="m",
                                      name=f"yp_{st}_{oc}")
                    emit_ymms(yp, st, oc)
                    emit_yout(yp, st, oc)
                return run

            def drain_d(c):
                # final chunk's out-proj: batch matmuls densely over 4
                # concurrent PSUM regions (borrowing the idle score pool)
                # so the cold PE re-warms and copies/DMAs pipeline behind
                pairs = [(4 * c + st4, oc)
                         for st4 in range(4) for oc in range(2)]
                for base in range(0, 8, 4):
                    spt = sp_ps.tile([P, 2, CW], F32, tag="sp",
                                     name=f"yd_{base}")
                    regs = [spt[:, 0, :], spt[:, 1, :],
                            misc_ps.tile([P, CW], F32, tag="m",
                                         name=f"ydm_{base}_0"),
                            misc_ps.tile([P, CW], F32, tag="m",
                                         name=f"ydm_{base}_1")]
                    for r, (st, oc) in zip(regs, pairs[base:base + 4]):
                        emit_ymms(r, st, oc)
                    for r, (st, oc) in zip(regs, pairs[base:base + 4]):
                        emit_yout(r, st, oc)

            # ---------------- prologue ----------------
            for c in range(CH):
                run_all(kq_proj_jobs(c, "k"))
            run_all(kq_proj_jobs(0, "q"))

            pending = deque()

            def pump(budget):
                while pending:
                    cost, fn = pending[0]
                    if cost > budget:
                        break
                    pending.popleft()
                    fn()
                    budget -= cost

            # ---------------- the ScalarE-bound pipeline ----------------
            for t in range(2 * CH):
                c, p = divmod(t, 2)
                csl = ds(CW * c, CW)
                if t == 0:
                    # v projections ride the pipeline; pass 0 uses a deep
                    # PV skew so they finish before their pvw is consumed
                    for cc in range(CH):
                        pending.extend(v_proj_jobs(cc))
                    pending.extend(kq_proj_jobs(1, "q"))
                if p == 0 and c + 2 < CH:
                    pending.extend(kq_proj_jobs(c + 2, "q"))

                skew = 7 if t == 0 else 1
                xaugs = [xa_ps.tile([P, 4, DK + 1], F32, tag="xa",
                                    name=f"xa_{c}_{p}_{i}")
                         for i in range(2)]
                pts = {}

                def emit_pv(sk, c=c, p=p, xaugs=xaugs, pts=pts):
                    pt = pts.pop(sk)
                    for hh in range(2):
                        for m in range(4):
                            # xaug[hh] = one bank shared by 4 m-groups:
                            # bank-wide clear only on the first matmul
                            nc.tensor.matmul(
                                xaugs[hh][:, m, :],
                                pt[:, hh, ds(P * m, P)],
                                pvw_sb[:, sk, 2 * p + hh, :],
                                start=(sk == 0 and m == 0),
                                stop=(sk == ST - 1))

                for sk in range(ST):
                    sp = sp_ps.tile([P, 2, CW], F32, tag="sp",
                                    name=f"sp_{t}_{sk}")
                    for hh in range(2):
                        nc.tensor.matmul(
                            sp[:, hh, :],
                            kt_sb[64 * hh:64 * hh + 64, p, ts(sk, P)],
                            qt_sb[64 * hh:64 * hh + 64, p, csl],
                            start=True, stop=True,
                            tile_position=(64 * hh, 0))
                    pt = ptp.tile([P, 2, CW], BF16, tag="pt",
                                  name=f"pt_{t}_{sk}")
                    pts[sk] = pt
                    nc.scalar.activation(pt[:], sp[:], AF.Exp, scale=0.125)
                    if sk >= skew:
                        emit_pv(sk - skew)
                    # let the exp backlog build before inserting jobs so
                    # ScalarE never starves at pass starts
                    if sk >= (4 if t == 0 else 2):
                        pump(1.7 if t == 0 else 0.9)
                for sk in range(ST - skew, ST):
                    emit_pv(sk)

                # ---- normalize; transposes are deferred jobs ----
                for hh in range(2):
                    rc = rcpp.tile([P, 4], F32, tag="rc",
                                   name=f"rc_{c}_{p}_{hh}")
                    nc.vector.reciprocal(rc[:], xaugs[hh][:, :, DK:DK + 1])
                    stg = stgp.tile([P, 4, DK], BF16, tag="stg",
                                    name=f"stg_{c}_{p}_{hh}")
                    for m in range(4):
                        nc.vector.tensor_scalar_mul(
                            stg[:, m, :], xaugs[hh][:, m, 0:DK],
                            rc[:, m:m + 1])
                    for i in range(2):
                        pending.append((0.3, transpose_job(c, p, hh, i, stg)))
                if p == 1 and c < CH - 1:
                    for st4 in range(4):
                        for oc in range(2):
                            pending.append(
                                (0.7, ytile_job(4 * c + st4, oc)))

            while pending:
                pending.popleft()[1]()
            drain_d(CH - 1)

    if apply_waitfix:
        _split_waits(nc, cap=1)
    return nc


_program_cache = {}


def get_program():
    if "nc" not in _program_cache:
        _program_cache["nc"] = build_program()
    return _program_cache["nc"]


def _warr(wT):
    """[D', M] -> [P, (D'/P)*M] with 4KB-contiguous per-partition rows."""
    dp, m = wT.shape
    kt = dp // P
    return np.ascontiguousarray(
        wT.reshape(kt, P, m).transpose(1, 0, 2).reshape(P, kt * m)
    ).astype(BF_NP)


def make_in_maps(Q, K, V, Wq, bq, Wk, bk, Wv, bv, Wo, bo):
    Q = np.asarray(Q, dtype=np.float32)
    K = np.asarray(K, dtype=np.float32)
    V = np.asarray(V, dtype=np.float32)
    Wq = np.asarray(Wq, dtype=np.float32)
    Wk = np.asarray(Wk, dtype=np.float32)
    Wv = np.asarray(Wv, dtype=np.float32)
    Wo = np.asarray(Wo, dtype=np.float32)
    bq = np.asarray(bq, dtype=np.float32)
    bk = np.asarray(bk, dtype=np.float32)
    bv = np.asarray(bv, dtype=np.float32)
    bo = np.asarray(bo, dtype=np.float32)

    def bf(a):
        return np.ascontiguousarray(a).astype(BF_NP)

    xt = {b: {
        "q": bf(Q[b].T),
        "k": bf(K[b].T),
        "v": bf(V[b].T),
    } for b in range(B)}

    ones1 = np.ones((1, P), dtype=np.float32)
    identm = np.eye(P, dtype=np.float32).astype(BF_NP)
    zero_bo = np.zeros((1, D), dtype=np.float32)
    bo_row = np.ascontiguousarray(bo.reshape(1, D))

    in_maps = []
    for c in range(NCORES):
        b, g = divmod(c, GROUPS)
        hs = slice(DH * g, DH * (g + 1))
        in_maps.append({
            "xqt": xt[b]["q"],
            "xkt": xt[b]["k"],
            "xvt": xt[b]["v"],
            "wq_h": _warr(Wq[hs, :].T),
            "wk_h": _warr(Wk[hs, :].T),
            "wv_h": _warr(Wv[hs, :].T),
            "wo_h": _warr(Wo[:, hs].T),
            "bq2": np.ascontiguousarray(bq[hs].reshape(2, P).T),
            "bk2": np.ascontiguousarray(bk[hs].reshape(2, P).T),
            "bvr": np.ascontiguousarray(bv[hs].reshape(1, DH)),
            "bo_eff": bo_row if g == 0 else zero_bo,
            "onesd": ones1,
            "identd": identm,
        })
    return in_maps


def combine_outputs(results):
    """results: list of 8 per-core dicts with 'y' [S, D] partials."""
    out = np.zeros((B, S, D), dtype=np.float32)
    for c, res in enumerate(results):
        b = c // GROUPS
        out[b] += res["y"]
    return out


def kernel(**inputs) -> np.ndarray:
    nc = get_program()
    in_maps = make_in_maps(**inputs)
    res = run_bass_kernel_spmd(nc, in_maps, core_ids=list(range(NCORES)))
    return combine_outputs(res.results)
